# revision 1
# baseline (speedup 1.0000x reference)
"""Trainium2 Bass kernel for:
    tanh( (x0*x1 + sin(x2)) * exp(-|x3|) + x4 / (x5*x5 + exp(x6)) - x7 )
over inputs (8388608, 8) f32, data-parallel over 8 NeuronCores.

Design notes:
  - Rows sharded 8-way across cores (pure data parallel).
  - Per core: 1,048,576 rows -> 16 tiles of (128 partitions x 512 rows).
    Each tile's input is a contiguous 2MB DRAM block, DMA'd as
    (128, 4096) f32; per-variable views are stride-8 APs in the free dim.
  - ACT (ScalarE) table sets: `sin` only coexists with tanh/abs/square in
    the silu/trig sets; `exp` lives in exp_and_others (with tanh).
    Table switches cost ~2.7us, so tiles are processed in batches of B=4:
    all sins of a batch first (one set), then all exp/tanh work (other
    set) -> 2 switches per batch instead of 2 per tile.  Explicit
    same-engine ordering edges keep the scheduler from interleaving.
  - Division via the custom-DVE reciprocal_approx_fast (~51 ULP, 1 op).
  - abs(x3) (via abs_max(x,0)) and x5*x5 run on GPSIMD to off-load the
    two hottest engines (DVE/ACT).
"""

import numpy as np

import concourse.bass as bass
import concourse.bacc as bacc
import concourse.mybir as mybir
from concourse.tile import TileContext
from concourse.tile_rust import add_dep_helper
from concourse import bass_utils

N_ROWS = 8_388_608
N_VARS = 8
N_CORES = 8
ROWS_PER_CORE = N_ROWS // N_CORES  # 1_048_576
P = 128          # SBUF partitions
F = 512          # rows per partition per tile
TILE_ROWS = P * F                  # 65_536
N_TILES = ROWS_PER_CORE // TILE_ROWS  # 16
B = 4            # tiles per ACT-table batch

F32 = mybir.dt.float32
AF = mybir.ActivationFunctionType
OP = mybir.AluOpType


def build_bass(dep_edges: bool = True, use_gpsimd: bool = True,
               n_tiles: int = N_TILES, b: int = B,
               k_iters: int = 1, loop_iters: int = 1,
               ablate: str = "none",
               gps_ops: tuple = (), deep_bufs: bool = False) -> bass.Bass:
    """ablate: 'none' | 'dma' (no compute) | 'nodve' | 'noact' —
    wrong results, used only to attribute time between engines."""
    import contextlib
    nc = bacc.Bacc("TRN2", debug=False, num_devices=N_CORES)
    x = nc.dram_tensor("x", [ROWS_PER_CORE, N_VARS], F32, kind="ExternalInput").ap()
    y = nc.dram_tensor("y", [ROWS_PER_CORE], F32, kind="ExternalOutput").ap()

    # deep_bufs: shrink input prefetch by one slot to afford 4-deep
    # buffering on the DVE-chain tiles (more tiles' chains in flight).
    inp_bufs = b + 1 if deep_bufs else b + 2
    dve_bufs = 4 if deep_bufs else 3
    with TileContext(nc) as tc:
        with (
            tc.tile_pool(name="inp", bufs=inp_bufs) as inp_pool,
            tc.tile_pool(name="sinp", bufs=b + 1 if deep_bufs else b + 2) as sin_pool,
            tc.tile_pool(name="tmp", bufs=3) as tmp_pool,
            (tc.For_i(0, loop_iters, 1) if loop_iters > 1
             else contextlib.nullcontext()),
        ):
            prev_batch_last_tanh = None
            for batch_start in [s for _ in range(k_iters)
                                for s in range(0, n_tiles, b)]:
                batch = list(range(batch_start, min(batch_start + b, n_tiles)))

                # ---- Phase S: load inputs, sin(x2) (sin table set) ----
                staged = []
                sin_insts = []
                for t in batch:
                    r0, r1 = t * TILE_ROWS, (t + 1) * TILE_ROWS
                    xt = inp_pool.tile([P, F * N_VARS], F32, name=f"xt{t}", tag="xt")
                    nc.sync.dma_start(
                        out=xt,
                        in_=x[r0:r1, :].rearrange("(p f) v -> p (f v)", p=P),
                    )
                    xv = xt.rearrange("p (f v) -> p f v", v=N_VARS)
                    if ablate == "dma":
                        nc.sync.dma_start(
                            out=y[r0:r1].rearrange("(p f) -> p f", p=P),
                            in_=xt[:, 0:F],
                        )
                        continue
                    st = sin_pool.tile([P, F], F32, name=f"st{t}", tag="st")
                    # ACT's sin spline is only accurate on [-pi, pi]; inputs
                    # reach |x2|~5.5, so wrap by one period first (DVE).
                    wr = sin_pool.tile([P, F], F32, name=f"wr{t}", tag="wr")
                    if ablate != "nodve":
                        nc.vector.add_range_wrap(
                            out=wr, in_=xv[:, :, 2], shift=0.0,
                            bound=float(np.pi), period=float(2 * np.pi),
                        )
                    si = None
                    if ablate != "noact":
                        src = xv[:, :, 2] if ablate == "nodve" else wr
                        si = nc.scalar.activation(st, src, AF.Sin)
                        if dep_edges and prev_batch_last_tanh is not None:
                            # keep ACT phases contiguous across batches
                            add_dep_helper(si.ins, prev_batch_last_tanh, False,
                                           "act-set phase order")
                        sin_insts.append(si.ins)
                    staged.append((t, xt, xv, st, wr))

                last_sin = sin_insts[-1] if sin_insts else None
                if ablate == "dma":
                    continue

                # ---- Phase E: everything else (exp_and_others set) ----
                for t, xt, xv, st, wr in staged:
                    r0, r1 = t * TILE_ROWS, (t + 1) * TILE_ROWS
                    def dtile(nm):
                        return tmp_pool.tile([P, F], F32, name=f"{nm}{t}",
                                             tag=nm, bufs=dve_bufs)
                    a = dtile("a")
                    bb = dtile("bb")
                    cc = tmp_pool.tile([P, F], F32, name=f"cc{t}", tag="cc")
                    e = tmp_pool.tile([P, F], F32, name=f"e{t}", tag="e")
                    f = dtile("f")
                    sq = tmp_pool.tile([P, F], F32, name=f"sq{t}", tag="sq")
                    e6 = tmp_pool.tile([P, F], F32, name=f"e6{t}", tag="e6")
                    d = dtile("d")
                    rc = dtile("rc")
                    q = dtile("q")
                    r = dtile("r")
                    u = dtile("u")
                    o = tmp_pool.tile([P, F], F32, name=f"o{t}", tag="o")

                    # GPSIMD: x5*x5 — the same-AP strided mult is cheap on
                    # Pool (~0.2us measured); copies there are NOT (~5us).
                    nc.gpsimd.tensor_tensor(
                        out=sq, in0=xv[:, :, 5], in1=xv[:, :, 5], op=OP.mult)

                    # ACT: cc=|x3| (Abs is in every table set), e=exp(-cc),
                    # e6=exp(x6)   (exp_and_others)
                    nc.scalar.activation(cc, xv[:, :, 3], AF.Abs)
                    i1 = nc.scalar.activation(e, cc, AF.Exp, scale=-1.0)
                    i2 = nc.scalar.activation(e6, xv[:, :, 6], AF.Exp)
                    if dep_edges and last_sin is not None:
                        for bi in (i1, i2):
                            add_dep_helper(bi.ins, last_sin, False,
                                           "act-set phase order")

                    # DVE chain (ops listed in gps_ops run on GPSIMD instead)
                    def eng_for(nm):
                        return nc.gpsimd if nm in gps_ops else nc.vector
                    eng_for("a").tensor_tensor(out=a, in0=xv[:, :, 0],
                                               in1=xv[:, :, 1],
                                               op=OP.mult)       # x0*x1
                    eng_for("bb").tensor_add(out=bb, in0=a, in1=st)
                    eng_for("f").tensor_tensor(out=f, in0=bb, in1=e,
                                               op=OP.mult)
                    eng_for("d").tensor_add(out=d, in0=sq, in1=e6)
                    nc.vector.reciprocal_approx_fast(out=rc, in_=d)
                    eng_for("q").tensor_tensor(out=q, in0=xv[:, :, 4], in1=rc,
                                               op=OP.mult)       # q = x4/d
                    eng_for("r").tensor_add(out=r, in0=f, in1=q)
                    eng_for("u").tensor_tensor(out=u, in0=r, in1=xv[:, :, 7],
                                               op=OP.subtract)

                    i3 = nc.scalar.activation(o, u, AF.Tanh)
                    if dep_edges and last_sin is not None:
                        add_dep_helper(i3.ins, last_sin, False,
                                       "act-set phase order")
                    prev_batch_last_tanh = i3.ins

                    nc.sync.dma_start(
                        out=y[r0:r1].rearrange("(p f) -> p f", p=P),
                        in_=o,
                    )
    nc.compile()
    return nc


_BUILT = None


def _get_built():
    global _BUILT
    if _BUILT is None:
        _BUILT = build_bass()
    return _BUILT


def run_spmd(inputs: np.ndarray, **kwargs) -> tuple[np.ndarray, object]:
    """Shard, run on 8 cores, gather.  Returns (full output, BassKernelResults).

    The axon-tunneled devices occasionally wedge transiently
    (NRT_EXEC_UNIT_UNRECOVERABLE); one retry after a pause usually
    recovers, so don't fail the whole run on the first error.
    """
    import time as _time
    x = np.ascontiguousarray(np.asarray(inputs, dtype=np.float32))
    assert x.shape == (N_ROWS, N_VARS), x.shape
    shards = x.reshape(N_CORES, ROWS_PER_CORE, N_VARS)
    in_maps = [{"x": np.ascontiguousarray(shards[i])} for i in range(N_CORES)]
    nc = _get_built()
    last_exc = None
    for attempt in range(3):
        try:
            res = bass_utils.run_bass_kernel_spmd(
                nc, in_maps, core_ids=list(range(N_CORES)), **kwargs
            )
            break
        except Exception as exc:  # transient device wedge — retry
            last_exc = exc
            _time.sleep(10 * (attempt + 1))
    else:
        raise last_exc
    out = np.concatenate([r["y"].reshape(-1) for r in res.results], axis=0)
    return out, res


def kernel(inputs: np.ndarray) -> np.ndarray:
    out, _ = run_spmd(inputs)
    return out



# revision 2
# speedup vs baseline: 19.8517x; 19.8517x over previous
"""Trainium2 Bass kernel for:
    tanh( (x0*x1 + sin(x2)) * exp(-|x3|) + x4 / (x5*x5 + exp(x6)) - x7 )
over inputs (8388608, 8) f32, data-parallel over 8 NeuronCores.

Design notes (see per-engine cost model in the TRN2 docs):
  - Rows sharded 8-way across cores (pure data parallel).
  - Per core: 1,048,576 rows -> 16 tiles of (128 partitions x 512 rows).
    Each tile's input is one contiguous 2MB HBM->SBUF DMA ([128, 4096]
    f32); per-variable views are stride-8 APs in the free dim.  2MB
    transfers with 6-deep prefetch keep the DMA engines ~92% occupied;
    larger chunks measured slower end-to-end (coarser dependency
    granularity hurts overlap more than per-DMA overhead saves).
  - Output DMAs merged in pairs (out_group=2): tanh results for two
    consecutive tiles land in one [128, 1024] SBUF tile, written by a
    single DMA via a 3D AP ("(g p f) -> p g f").
  - ACT (ScalarE) work: sin (trig/silu table sets) + abs/exp/tanh
    (exp_and_others).  Table switches cost ~2.7us, but measurements
    show scheduler freedom beats switch-minimizing ordering edges:
    forced same-engine phase edges cause head-of-line stalls that cost
    more than the extra ACT_TABLE_LOADs, so no ordering edges are
    emitted (dep_edges off).  tanh emission is still deferred by one
    batch (pipe_tanh) so a tanh never sits between a tile's exps and
    the DVE chain that feeds it.
  - sin needs range reduction (ACT spline accurate on [-pi, pi];
    inputs reach |x2|~5.5): one fused DVE add_range_wrap per tile.
  - Division via the custom-DVE reciprocal_approx_fast (~51 ULP).
  - x5*x5 runs on GPSIMD to offload DVE (the busiest compute engine).
"""

import contextlib
import time

import numpy as np

import concourse.bass as bass
import concourse.bacc as bacc
import concourse.mybir as mybir
from concourse.tile import TileContext
from concourse import bass_utils

N_ROWS = 8_388_608
N_VARS = 8
N_CORES = 8
ROWS_PER_CORE = N_ROWS // N_CORES  # 1_048_576
P = 128          # SBUF partitions
F = 512          # rows per partition per tile
TILE_ROWS = P * F                  # 65_536
N_TILES = ROWS_PER_CORE // TILE_ROWS  # 16
B = 4            # tiles per emission batch
G = 2            # tiles per merged output DMA

F32 = mybir.dt.float32
AF = mybir.ActivationFunctionType
OP = mybir.AluOpType


def emit_body(nc, tc):
    x = nc.dram_tensor("x", [ROWS_PER_CORE, N_VARS], F32,
                       kind="ExternalInput").ap()
    y = nc.dram_tensor("y", [ROWS_PER_CORE], F32, kind="ExternalOutput").ap()

    with (
        tc.tile_pool(name="inp", bufs=B + 2) as inp_pool,
        tc.tile_pool(name="sinp", bufs=5) as sin_pool,
        tc.tile_pool(name="tmp", bufs=3) as tmp_pool,
        tc.tile_pool(name="og", bufs=2) as og_pool,
    ):
        og_tiles = {}
        pending = []  # (t, u) awaiting tanh + output DMA

        def emit_tanh(t, u):
            grp, g = divmod(t, G)
            if grp not in og_tiles:
                og_tiles[grp] = og_pool.tile([P, G * F], F32,
                                             name=f"og{grp}", tag="og")
            o = og_tiles[grp][:, g * F:(g + 1) * F]
            nc.scalar.activation(o, u, AF.Tanh)
            if g == G - 1:
                R0 = grp * G * TILE_ROWS
                nc.sync.dma_start(
                    out=y[R0:R0 + G * TILE_ROWS].rearrange(
                        "(g p f) -> p g f", g=G, p=P),
                    in_=og_tiles[grp].rearrange("p (g f) -> p g f", g=G),
                )

        for batch_start in range(0, N_TILES, B):
            batch = range(batch_start, min(batch_start + B, N_TILES))

            staged = []
            for t in batch:
                r0 = t * TILE_ROWS
                xt = inp_pool.tile([P, F * N_VARS], F32, name=f"xt{t}",
                                   tag="xt")
                nc.sync.dma_start(
                    out=xt,
                    in_=x[r0:r0 + TILE_ROWS, :].rearrange(
                        "(p f) v -> p (f v)", p=P),
                )
                xv = xt.rearrange("p (f v) -> p f v", v=N_VARS)
                st = sin_pool.tile([P, F], F32, name=f"st{t}", tag="st")
                # ACT's sin spline is only accurate on [-pi, pi]; inputs
                # reach |x2|~5.5, so wrap by one period first (DVE).
                wr = sin_pool.tile([P, F], F32, name=f"wr{t}", tag="wr")
                nc.vector.add_range_wrap(
                    out=wr, in_=xv[:, :, 2], shift=0.0,
                    bound=float(np.pi), period=float(2 * np.pi),
                )
                nc.scalar.activation(st, wr, AF.Sin)
                staged.append((t, xv, st))

            # tanhs of the previous batch (tanh lives in every relevant
            # ACT table set, so it never forces an extra table load)
            for (pt, pu) in pending:
                emit_tanh(pt, pu)
            pending.clear()

            for t, xv, st in staged:
                def dtile(nm):
                    return tmp_pool.tile([P, F], F32, name=f"{nm}{t}",
                                         tag=nm)
                a = dtile("a")
                bb = dtile("bb")
                cc = dtile("cc")
                e = dtile("e")
                f = dtile("f")
                sq = dtile("sq")
                e6 = dtile("e6")
                d = dtile("d")
                rc = dtile("rc")
                q = dtile("q")
                r = dtile("r")
                u = tmp_pool.tile([P, F], F32, name=f"u{t}", tag="u",
                                  bufs=B + 3)

                nc.gpsimd.tensor_tensor(
                    out=sq, in0=xv[:, :, 5], in1=xv[:, :, 5], op=OP.mult)

                nc.scalar.activation(cc, xv[:, :, 3], AF.Abs)
                nc.scalar.activation(e, cc, AF.Exp, scale=-1.0)
                nc.scalar.activation(e6, xv[:, :, 6], AF.Exp)

                nc.vector.tensor_tensor(out=a, in0=xv[:, :, 0],
                                        in1=xv[:, :, 1], op=OP.mult)
                nc.vector.tensor_add(out=bb, in0=a, in1=st)
                nc.vector.tensor_tensor(out=f, in0=bb, in1=e, op=OP.mult)
                nc.vector.tensor_add(out=d, in0=sq, in1=e6)
                nc.vector.reciprocal_approx_fast(out=rc, in_=d)
                nc.vector.tensor_tensor(out=q, in0=xv[:, :, 4], in1=rc,
                                        op=OP.mult)
                nc.vector.tensor_add(out=r, in0=f, in1=q)
                nc.vector.tensor_tensor(out=u, in0=r, in1=xv[:, :, 7],
                                        op=OP.subtract)
                pending.append((t, u))

        for (pt, pu) in pending:
            emit_tanh(pt, pu)
        pending.clear()


def build_bass(loop_iters: int = 1) -> bass.Bass:
    """loop_iters > 1 wraps the body in a hardware For_i (bench only)."""
    nc = bacc.Bacc("TRN2", debug=False, num_devices=N_CORES)
    with TileContext(nc) as tc:
        with (tc.For_i(0, loop_iters, 1) if loop_iters > 1
              else contextlib.nullcontext()):
            emit_body(nc, tc)
    nc.compile()
    return nc


_BUILT = None


def _get_built():
    global _BUILT
    if _BUILT is None:
        _BUILT = build_bass()
    return _BUILT


def run_spmd(inputs: np.ndarray, **kwargs):
    """Shard, run on 8 cores, gather.  Returns (full output, results).

    The axon-tunneled devices occasionally wedge transiently
    (NRT_EXEC_UNIT_UNRECOVERABLE); retry after a pause.
    """
    x = np.ascontiguousarray(np.asarray(inputs, dtype=np.float32))
    assert x.shape == (N_ROWS, N_VARS), x.shape
    shards = x.reshape(N_CORES, ROWS_PER_CORE, N_VARS)
    in_maps = [{"x": np.ascontiguousarray(shards[i])} for i in range(N_CORES)]
    nc = _get_built()
    last_exc = None
    for attempt in range(3):
        try:
            res = bass_utils.run_bass_kernel_spmd(
                nc, in_maps, core_ids=list(range(N_CORES)), **kwargs
            )
            break
        except Exception as exc:  # transient device wedge — retry
            last_exc = exc
            time.sleep(10 * (attempt + 1))
    else:
        raise last_exc
    out = np.concatenate([r["y"].reshape(-1) for r in res.results], axis=0)
    return out, res


def kernel(inputs: np.ndarray) -> np.ndarray:
    out, _ = run_spmd(inputs)
    return out


# revision 3
# speedup vs baseline: 19.9657x; 1.0057x over previous
"""Trainium2 Bass kernel for:
    tanh( (x0*x1 + sin(x2)) * exp(-|x3|) + x4 / (x5*x5 + exp(x6)) - x7 )
over inputs (8388608, 8) f32, data-parallel over 8 NeuronCores.

Design notes (see per-engine cost model in the TRN2 docs):
  - Rows sharded 8-way across cores (pure data parallel).
  - Per core: 1,048,576 rows -> 16 tiles of (128 partitions x 512 rows).
    Each tile's input is one contiguous 2MB HBM->SBUF DMA ([128, 4096]
    f32); per-variable views are stride-8 APs in the free dim.  2MB
    transfers with 6-deep prefetch keep the DMA engines ~92% occupied;
    larger chunks measured slower end-to-end (coarser dependency
    granularity hurts overlap more than per-DMA overhead saves).
  - Output DMAs merged in pairs (out_group=2): tanh results for two
    consecutive tiles land in one [128, 1024] SBUF tile, written by a
    single DMA via a 3D AP ("(g p f) -> p g f").  The final two tiles
    write per-tile DMAs instead, so the last store starts the moment
    its own tanh finishes (shorter pipeline drain).
  - ACT (ScalarE) work: sin (trig/silu table sets) + abs/exp/tanh
    (exp_and_others).  Table switches cost ~2.7us, but measurements
    show scheduler freedom beats switch-minimizing ordering edges:
    forced same-engine phase edges cause head-of-line stalls that cost
    more than the extra ACT_TABLE_LOADs, so no ordering edges are
    emitted (dep_edges off).  tanh emission is still deferred by one
    batch (pipe_tanh) so a tanh never sits between a tile's exps and
    the DVE chain that feeds it.
  - sin needs range reduction (ACT spline accurate on [-pi, pi];
    inputs reach |x2|~5.5): one fused DVE add_range_wrap per tile.
  - |x3| as an int32 bitwise-and (sign-bit clear) on DVE via AP
    bitcast: exact, and one op cheaper on ACT than AF.Abs.
  - Division via the custom-DVE reciprocal_approx_fast (~51 ULP).
  - x5*x5 runs on GPSIMD to offload DVE (the busiest compute engine).
"""

import contextlib
import time

import numpy as np

import concourse.bass as bass
import concourse.bacc as bacc
import concourse.mybir as mybir
from concourse.tile import TileContext
from concourse import bass_utils

N_ROWS = 8_388_608
N_VARS = 8
N_CORES = 8
ROWS_PER_CORE = N_ROWS // N_CORES  # 1_048_576
P = 128          # SBUF partitions
F = 512          # rows per partition per tile
TILE_ROWS = P * F                  # 65_536
N_TILES = ROWS_PER_CORE // TILE_ROWS  # 16
B = 4            # tiles per emission batch
G = 2            # tiles per merged output DMA

F32 = mybir.dt.float32
AF = mybir.ActivationFunctionType
OP = mybir.AluOpType


def emit_body(nc, tc):
    x = nc.dram_tensor("x", [ROWS_PER_CORE, N_VARS], F32,
                       kind="ExternalInput").ap()
    y = nc.dram_tensor("y", [ROWS_PER_CORE], F32, kind="ExternalOutput").ap()

    with (
        tc.tile_pool(name="inp", bufs=B + 2) as inp_pool,
        tc.tile_pool(name="sinp", bufs=4) as sin_pool,
        tc.tile_pool(name="tmp", bufs=3) as tmp_pool,
        tc.tile_pool(name="og", bufs=2) as og_pool,
    ):
        og_tiles = {}
        pending = []  # (t, u) awaiting tanh + output DMA

        def emit_tanh(t, u):
            if t >= N_TILES - 2:
                # final tiles: per-tile output DMA (shorter drain)
                o = tmp_pool.tile([P, F], F32, name=f"o{t}", tag="og")
                nc.scalar.activation(o, u, AF.Tanh)
                r0 = t * TILE_ROWS
                nc.sync.dma_start(
                    out=y[r0:r0 + TILE_ROWS].rearrange("(p f) -> p f", p=P),
                    in_=o,
                )
                return
            grp, g = divmod(t, G)
            if grp not in og_tiles:
                og_tiles[grp] = og_pool.tile([P, G * F], F32,
                                             name=f"og{grp}", tag="og")
            o = og_tiles[grp][:, g * F:(g + 1) * F]
            nc.scalar.activation(o, u, AF.Tanh)
            if g == G - 1:
                R0 = grp * G * TILE_ROWS
                nc.sync.dma_start(
                    out=y[R0:R0 + G * TILE_ROWS].rearrange(
                        "(g p f) -> p g f", g=G, p=P),
                    in_=og_tiles[grp].rearrange("p (g f) -> p g f", g=G),
                )

        for batch_start in range(0, N_TILES, B):
            batch = range(batch_start, min(batch_start + B, N_TILES))

            staged = []
            for t in batch:
                r0 = t * TILE_ROWS
                xt = inp_pool.tile([P, F * N_VARS], F32, name=f"xt{t}",
                                   tag="xt")
                nc.sync.dma_start(
                    out=xt,
                    in_=x[r0:r0 + TILE_ROWS, :].rearrange(
                        "(p f) v -> p (f v)", p=P),
                )
                xv = xt.rearrange("p (f v) -> p f v", v=N_VARS)
                st = sin_pool.tile([P, F], F32, name=f"st{t}", tag="st")
                # ACT's sin spline is only accurate on [-pi, pi]; inputs
                # reach |x2|~5.5, so wrap by one period first (DVE).
                wr = sin_pool.tile([P, F], F32, name=f"wr{t}", tag="wr")
                nc.vector.add_range_wrap(
                    out=wr, in_=xv[:, :, 2], shift=0.0,
                    bound=float(np.pi), period=float(2 * np.pi),
                )
                nc.scalar.activation(st, wr, AF.Sin)
                staged.append((t, xv, st))

            # tanhs of the previous batch (tanh lives in every relevant
            # ACT table set, so it never forces an extra table load)
            for (pt, pu) in pending:
                emit_tanh(pt, pu)
            pending.clear()

            for t, xv, st in staged:
                def dtile(nm):
                    return tmp_pool.tile([P, F], F32, name=f"{nm}{t}",
                                         tag=nm)
                a = dtile("a")
                bb = dtile("bb")
                cc = dtile("cc")
                e = dtile("e")
                f = dtile("f")
                sq = dtile("sq")
                e6 = dtile("e6")
                d = dtile("d")
                rc = dtile("rc")
                q = dtile("q")
                r = dtile("r")
                u = tmp_pool.tile([P, F], F32, name=f"u{t}", tag="u",
                                  bufs=B + 3)

                nc.gpsimd.tensor_tensor(
                    out=sq, in0=xv[:, :, 5], in1=xv[:, :, 5], op=OP.mult)

                # |x3|: clear the sign bit on an int32 view (exact)
                I32 = mybir.dt.int32
                nc.vector.tensor_scalar(
                    out=cc.bitcast(I32), in0=xv[:, :, 3].bitcast(I32),
                    scalar1=0x7FFFFFFF, scalar2=None, op0=OP.bitwise_and)
                nc.scalar.activation(e, cc, AF.Exp, scale=-1.0)
                nc.scalar.activation(e6, xv[:, :, 6], AF.Exp)

                nc.vector.tensor_tensor(out=a, in0=xv[:, :, 0],
                                        in1=xv[:, :, 1], op=OP.mult)
                nc.vector.tensor_add(out=bb, in0=a, in1=st)
                nc.vector.tensor_tensor(out=f, in0=bb, in1=e, op=OP.mult)
                nc.vector.tensor_add(out=d, in0=sq, in1=e6)
                nc.vector.reciprocal_approx_fast(out=rc, in_=d)
                nc.vector.tensor_tensor(out=q, in0=xv[:, :, 4], in1=rc,
                                        op=OP.mult)
                nc.vector.tensor_add(out=r, in0=f, in1=q)
                nc.vector.tensor_tensor(out=u, in0=r, in1=xv[:, :, 7],
                                        op=OP.subtract)
                pending.append((t, u))

        for (pt, pu) in pending:
            emit_tanh(pt, pu)
        pending.clear()


def build_bass(loop_iters: int = 1) -> bass.Bass:
    """loop_iters > 1 wraps the body in a hardware For_i (bench only)."""
    nc = bacc.Bacc("TRN2", debug=False, num_devices=N_CORES)
    with TileContext(nc) as tc:
        with (tc.For_i(0, loop_iters, 1) if loop_iters > 1
              else contextlib.nullcontext()):
            emit_body(nc, tc)
    nc.compile()
    return nc


_BUILT = None


def _get_built():
    global _BUILT
    if _BUILT is None:
        _BUILT = build_bass()
    return _BUILT


def run_spmd(inputs: np.ndarray, **kwargs):
    """Shard, run on 8 cores, gather.  Returns (full output, results).

    The axon-tunneled devices occasionally wedge transiently
    (NRT_EXEC_UNIT_UNRECOVERABLE); retry after a pause.
    """
    x = np.ascontiguousarray(np.asarray(inputs, dtype=np.float32))
    assert x.shape == (N_ROWS, N_VARS), x.shape
    shards = x.reshape(N_CORES, ROWS_PER_CORE, N_VARS)
    in_maps = [{"x": np.ascontiguousarray(shards[i])} for i in range(N_CORES)]
    nc = _get_built()
    last_exc = None
    for attempt in range(3):
        try:
            res = bass_utils.run_bass_kernel_spmd(
                nc, in_maps, core_ids=list(range(N_CORES)), **kwargs
            )
            break
        except Exception as exc:  # transient device wedge — retry
            last_exc = exc
            time.sleep(10 * (attempt + 1))
    else:
        raise last_exc
    out = np.concatenate([r["y"].reshape(-1) for r in res.results], axis=0)
    return out, res


def kernel(inputs: np.ndarray) -> np.ndarray:
    out, _ = run_spmd(inputs)
    return out


# revision 4
# speedup vs baseline: 20.0065x; 1.0020x over previous
"""Trainium2 Bass kernel for:
    tanh( (x0*x1 + sin(x2)) * exp(-|x3|) + x4 / (x5*x5 + exp(x6)) - x7 )
over inputs (8388608, 8) f32, data-parallel over 8 NeuronCores.

Design notes (see per-engine cost model in the TRN2 docs):
  - Rows sharded 8-way across cores (pure data parallel).
  - Per core: 1,048,576 rows -> 16 tiles of (128 partitions x 512 rows).
    Each tile's input is one contiguous 2MB HBM->SBUF DMA ([128, 4096]
    f32); per-variable views are stride-8 APs in the free dim.  2MB
    transfers with 6-deep prefetch keep the DMA engines ~92% occupied;
    larger chunks measured slower end-to-end (coarser dependency
    granularity hurts overlap more than per-DMA overhead saves).
  - Output DMAs merged in pairs (out_group=2): tanh results for two
    consecutive tiles land in one [128, 1024] SBUF tile, written by a
    single DMA via a 3D AP ("(g p f) -> p g f").  The final two tiles
    write per-tile DMAs instead, so the last store starts the moment
    its own tanh finishes (shorter pipeline drain).
  - ACT (ScalarE) work: sin (trig/silu table sets) + abs/exp/tanh
    (exp_and_others).  Table switches cost ~2.7us, but measurements
    show scheduler freedom beats switch-minimizing ordering edges:
    forced same-engine phase edges cause head-of-line stalls that cost
    more than the extra ACT_TABLE_LOADs, so no ordering edges are
    emitted (dep_edges off).  tanh emission is still deferred by one
    batch (pipe_tanh) so a tanh never sits between a tile's exps and
    the DVE chain that feeds it.
  - sin needs range reduction (ACT spline accurate on [-pi, pi];
    inputs reach |x2|~5.5): one fused DVE add_range_wrap per tile.
  - |x3| as an int32 bitwise-and (sign-bit clear) on DVE via AP
    bitcast: exact, and one op cheaper on ACT than AF.Abs.
  - Division via the custom-DVE reciprocal_approx_fast (~51 ULP).
  - x5*x5 runs on GPSIMD to offload DVE (the busiest compute engine).
"""

import contextlib
import time

import numpy as np

import concourse.bass as bass
import concourse.bacc as bacc
import concourse.mybir as mybir
from concourse.tile import TileContext
from concourse import bass_utils

N_ROWS = 8_388_608
N_VARS = 8
N_CORES = 8
ROWS_PER_CORE = N_ROWS // N_CORES  # 1_048_576
P = 128          # SBUF partitions
F = 512          # rows per partition per tile
TILE_ROWS = P * F                  # 65_536
N_TILES = ROWS_PER_CORE // TILE_ROWS  # 16
B = 6            # tiles per emission batch (b=6 measured best:
                 # sets the tanh-deferral distance under pipe_tanh)
G = 2            # tiles per merged output DMA

F32 = mybir.dt.float32
AF = mybir.ActivationFunctionType
OP = mybir.AluOpType


def emit_body(nc, tc):
    x = nc.dram_tensor("x", [ROWS_PER_CORE, N_VARS], F32,
                       kind="ExternalInput").ap()
    y = nc.dram_tensor("y", [ROWS_PER_CORE], F32, kind="ExternalOutput").ap()

    with (
        tc.tile_pool(name="inp", bufs=6) as inp_pool,
        tc.tile_pool(name="sinp", bufs=4) as sin_pool,
        tc.tile_pool(name="tmp", bufs=3) as tmp_pool,
        tc.tile_pool(name="og", bufs=2) as og_pool,
    ):
        og_tiles = {}
        pending = []  # (t, u) awaiting tanh + output DMA

        def emit_tanh(t, u):
            if t >= N_TILES - 2:
                # final tiles: per-tile output DMA (shorter drain)
                o = tmp_pool.tile([P, F], F32, name=f"o{t}", tag="og")
                nc.scalar.activation(o, u, AF.Tanh)
                r0 = t * TILE_ROWS
                nc.sync.dma_start(
                    out=y[r0:r0 + TILE_ROWS].rearrange("(p f) -> p f", p=P),
                    in_=o,
                )
                return
            grp, g = divmod(t, G)
            if grp not in og_tiles:
                og_tiles[grp] = og_pool.tile([P, G * F], F32,
                                             name=f"og{grp}", tag="og")
            o = og_tiles[grp][:, g * F:(g + 1) * F]
            nc.scalar.activation(o, u, AF.Tanh)
            if g == G - 1:
                R0 = grp * G * TILE_ROWS
                nc.sync.dma_start(
                    out=y[R0:R0 + G * TILE_ROWS].rearrange(
                        "(g p f) -> p g f", g=G, p=P),
                    in_=og_tiles[grp].rearrange("p (g f) -> p g f", g=G),
                )

        for batch_start in range(0, N_TILES, B):
            batch = range(batch_start, min(batch_start + B, N_TILES))

            staged = []
            for t in batch:
                r0 = t * TILE_ROWS
                xt = inp_pool.tile([P, F * N_VARS], F32, name=f"xt{t}",
                                   tag="xt")
                nc.sync.dma_start(
                    out=xt,
                    in_=x[r0:r0 + TILE_ROWS, :].rearrange(
                        "(p f) v -> p (f v)", p=P),
                )
                xv = xt.rearrange("p (f v) -> p f v", v=N_VARS)
                st = sin_pool.tile([P, F], F32, name=f"st{t}", tag="st")
                # ACT's sin spline is only accurate on [-pi, pi]; inputs
                # reach |x2|~5.5, so wrap by one period first (DVE).
                wr = sin_pool.tile([P, F], F32, name=f"wr{t}", tag="wr")
                nc.vector.add_range_wrap(
                    out=wr, in_=xv[:, :, 2], shift=0.0,
                    bound=float(np.pi), period=float(2 * np.pi),
                )
                nc.scalar.activation(st, wr, AF.Sin)
                staged.append((t, xv, st))

            # tanhs of the previous batch (tanh lives in every relevant
            # ACT table set, so it never forces an extra table load)
            for (pt, pu) in pending:
                emit_tanh(pt, pu)
            pending.clear()

            for t, xv, st in staged:
                def dtile(nm):
                    return tmp_pool.tile([P, F], F32, name=f"{nm}{t}",
                                         tag=nm)
                a = dtile("a")
                bb = dtile("bb")
                cc = dtile("cc")
                e = dtile("e")
                f = dtile("f")
                sq = dtile("sq")
                e6 = dtile("e6")
                d = dtile("d")
                rc = dtile("rc")
                q = dtile("q")
                r = dtile("r")
                u = tmp_pool.tile([P, F], F32, name=f"u{t}", tag="u",
                                  bufs=6)

                nc.gpsimd.tensor_tensor(
                    out=sq, in0=xv[:, :, 5], in1=xv[:, :, 5], op=OP.mult)

                # |x3|: clear the sign bit on an int32 view (exact)
                I32 = mybir.dt.int32
                nc.vector.tensor_scalar(
                    out=cc.bitcast(I32), in0=xv[:, :, 3].bitcast(I32),
                    scalar1=0x7FFFFFFF, scalar2=None, op0=OP.bitwise_and)
                nc.scalar.activation(e, cc, AF.Exp, scale=-1.0)
                nc.scalar.activation(e6, xv[:, :, 6], AF.Exp)

                nc.vector.tensor_tensor(out=a, in0=xv[:, :, 0],
                                        in1=xv[:, :, 1], op=OP.mult)
                nc.vector.tensor_add(out=bb, in0=a, in1=st)
                nc.vector.tensor_tensor(out=f, in0=bb, in1=e, op=OP.mult)
                nc.vector.tensor_add(out=d, in0=sq, in1=e6)
                nc.vector.reciprocal_approx_fast(out=rc, in_=d)
                nc.vector.tensor_tensor(out=q, in0=xv[:, :, 4], in1=rc,
                                        op=OP.mult)
                nc.vector.tensor_add(out=r, in0=f, in1=q)
                nc.vector.tensor_tensor(out=u, in0=r, in1=xv[:, :, 7],
                                        op=OP.subtract)
                pending.append((t, u))

        for (pt, pu) in pending:
            emit_tanh(pt, pu)
        pending.clear()


def build_bass(loop_iters: int = 1) -> bass.Bass:
    """loop_iters > 1 wraps the body in a hardware For_i (bench only)."""
    nc = bacc.Bacc("TRN2", debug=False, num_devices=N_CORES)
    with TileContext(nc) as tc:
        with (tc.For_i(0, loop_iters, 1) if loop_iters > 1
              else contextlib.nullcontext()):
            emit_body(nc, tc)
    nc.compile()
    return nc


_BUILT = None


def _get_built():
    global _BUILT
    if _BUILT is None:
        _BUILT = build_bass()
    return _BUILT


def run_spmd(inputs: np.ndarray, **kwargs):
    """Shard, run on 8 cores, gather.  Returns (full output, results).

    The axon-tunneled devices occasionally wedge transiently
    (NRT_EXEC_UNIT_UNRECOVERABLE); retry after a pause.
    """
    x = np.ascontiguousarray(np.asarray(inputs, dtype=np.float32))
    assert x.shape == (N_ROWS, N_VARS), x.shape
    shards = x.reshape(N_CORES, ROWS_PER_CORE, N_VARS)
    in_maps = [{"x": np.ascontiguousarray(shards[i])} for i in range(N_CORES)]
    nc = _get_built()
    last_exc = None
    for attempt in range(3):
        try:
            res = bass_utils.run_bass_kernel_spmd(
                nc, in_maps, core_ids=list(range(N_CORES)), **kwargs
            )
            break
        except Exception as exc:  # transient device wedge — retry
            last_exc = exc
            time.sleep(10 * (attempt + 1))
    else:
        raise last_exc
    out = np.concatenate([r["y"].reshape(-1) for r in res.results], axis=0)
    return out, res


def kernel(inputs: np.ndarray) -> np.ndarray:
    out, _ = run_spmd(inputs)
    return out
